# revision 7
# baseline (speedup 1.0000x reference)
"""Additive (Bahdanau) attention TRN2 Bass kernel.

Problem: nn_AdditiveAttention (B=64, S=2048, DH=DE=DA=1024, fp32).

    query  = decoder_hidden @ w_query            [B, DA]
    keys   = encoder_outputs @ w_key             [B, S, DA]
    scores = tanh(query[:,None,:] + keys) @ w_score   [B, S]
    weights = softmax(where(mask, scores, -1e9))      [B, S]
    context = weights @ encoder_outputs               [B, DE]

Sharding: data-parallel over batch. 8 cores x 8 batches/core; weights
replicated. No collectives.

Per-core plan (all matmuls in float32r = fp32 data, bf16-class PE rate):
  - keysT[DA, S] = w_key^T @ enc^T per batch, via PE on-chip transposes of
    enc (fp32 can't DMA-transpose). lhsT = w_key (natural layout).
  - tanh fused with +query on ACT (query chunk is a per-partition bias).
  - scores = w_score . tanh as an M=1 matmul over the DA partition dim.
  - softmax on a [1, S] row; exp+sum fused in one ACT op.
  - context via M=1 matmuls with naturally-laid-out enc tiles (K=S).
"""

import numpy as np

import concourse.bacc as bacc
import concourse.mybir as mybir
import concourse.tile as tile
from concourse.bass_utils import run_bass_kernel_spmd
from concourse.masks import make_identity

P = 128
B_FULL, S, D = 64, 2048, 1024
NCORES = 8
BSH = B_FULL // NCORES  # batches per core
NT = S // P             # s-tiles per batch (16)
KJ = D // P             # 128-row contraction blocks (8)
SCH = 512               # S-chunk width for keys/scores
NCH = S // SCH          # 4
MT = D // P             # DA m-tiles (8)
F32 = mybir.dt.float32
F32R = mybir.dt.float32r
U8 = mybir.dt.uint8
AF = mybir.ActivationFunctionType
ALU = mybir.AluOpType


def _r(ap):
    """View an fp32 AP as float32r for full-rate PE matmuls."""
    return ap.bitcast(F32R)


def build_nc(reps: int = 1):
    nc = bacc.Bacc("TRN2", target_bir_lowering=False, debug=False)
    dh = nc.dram_tensor("dh", [BSH, D], F32, kind="ExternalInput").ap()
    enc = nc.dram_tensor("enc", [BSH, S, D], F32R, kind="ExternalInput").ap()
    msk = nc.dram_tensor("msk", [BSH, S], U8, kind="ExternalInput").ap()
    wq = nc.dram_tensor("wq", [D, D], F32R, kind="ExternalInput").ap()
    wk = nc.dram_tensor("wk", [D, D], F32R, kind="ExternalInput").ap()
    ws = nc.dram_tensor("ws", [D], F32R, kind="ExternalInput").ap()
    ctx_out = nc.dram_tensor("ctx", [BSH, D], F32, kind="ExternalOutput").ap()
    wts_out = nc.dram_tensor("wts", [BSH, S], F32, kind="ExternalOutput").ap()

    with tile.TileContext(nc) as tc:
        with tc.tile_pool(name="const", bufs=1) as constp:
            ident = constp.tile([P, P], F32)
            make_identity(nc, ident[:])
            identr = constp.tile([P, P], F32R)
            nc.vector.tensor_copy(identr[:], ident[:])
            negbig = constp.tile([1, 1], F32)
            nc.gpsimd.memset(negbig[:], -1e9)

            wk_sb = constp.tile([P, KJ, D], F32R)
            nc.sync.dma_start(wk_sb[:], wk.rearrange("(ko ki) a -> ki ko a", ki=P))
            ws_sb = constp.tile([P, KJ], F32R)
            nc.sync.dma_start(ws_sb[:], ws.rearrange("(ko ki) -> ki ko", ki=P))
            dh_sb = constp.tile([BSH, D], F32)
            nc.sync.dma_start(dh_sb[:], dh)
            dhT_sb = constp.tile([P, KJ, BSH], F32R)
            q_sb = constp.tile([P, MT, BSH], F32)

            # ---- phase 0: Q^T[DA, B] = w_query^T @ dh^T ----
            with (
                tc.tile_pool(name="ph0", bufs=1) as ph0,
                tc.tile_pool(name="ph0ps", bufs=2, space="PSUM") as ph0ps,
            ):
                for k in range(KJ):
                    tp = ph0ps.tile([P, BSH], F32, tag="dhtp")
                    nc.tensor.transpose(
                        tp[:], dh_sb[:, k * P:(k + 1) * P], ident[:BSH, :BSH]
                    )
                    nc.vector.tensor_copy(dhT_sb[:, k, :], tp[:])
                wq_sb = ph0.tile([P, KJ, D], F32R, tag="wq")
                nc.sync.dma_start(wq_sb[:], wq.rearrange("(ko ki) a -> ki ko a", ki=P))
                for m in range(MT):
                    qp = ph0ps.tile([P, BSH], F32, tag="qps")
                    for k in range(KJ):
                        nc.tensor.matmul(
                            qp[:],
                            wq_sb[:, k, m * P:(m + 1) * P],
                            dhT_sb[:, k, :],
                            start=(k == 0),
                            stop=(k == KJ - 1),
                        )
                    nc.vector.tensor_copy(q_sb[:, m, :], qp[:])

            # ---- main per-batch pipeline ----
            with (
                tc.tile_pool(name="encp", bufs=21) as encp,
                tc.tile_pool(name="encTp", bufs=16) as encTp,
                tc.tile_pool(name="tanhp", bufs=4) as tanhp,
                tc.tile_pool(name="rows1", bufs=1) as rows1,
                tc.tile_pool(name="rows2", bufs=2) as rows2,
                tc.tile_pool(name="stats", bufs=2) as statsp,
                tc.tile_pool(name="keysps", bufs=2, space="PSUM") as keysps,
                tc.tile_pool(name="tpsps", bufs=3, space="PSUM") as tpsps,
                tc.tile_pool(name="scps", bufs=2, space="PSUM") as scps,
                tc.tile_pool(name="ctxps", bufs=1, space="PSUM") as ctxps,
            ):

                state = {}

                def emit_tail(b, enc_t, scrow):
                    # mask + softmax + weights-transpose + context for batch b
                    mrow = rows1.tile([1, S], U8, tag="mrow")
                    nc.sync.dma_start(mrow[:], msk[b:b + 1, :])
                    notm = rows1.tile([1, S], U8, tag="notm")
                    nc.vector.tensor_scalar(
                        notm[:], mrow[:], 0, None, ALU.is_equal
                    )
                    nc.vector.copy_predicated(
                        scrow[:], notm[:], negbig[:].to_broadcast((1, S))
                    )
                    mx = statsp.tile([1, 1], F32, tag="mx")
                    nc.vector.reduce_max(
                        mx[:], scrow[:], axis=mybir.AxisListType.X, negate=True
                    )
                    wrow = rows2.tile([1, S], F32, tag="wrow")
                    sm = statsp.tile([1, 1], F32, tag="sm")
                    nc.scalar.activation(
                        wrow[:], scrow[:], AF.Exp, bias=mx[:], scale=1.0,
                        accum_out=sm[:],
                    )
                    rec = statsp.tile([1, 1], F32, tag="rec")
                    nc.vector.reciprocal(rec[:], sm[:])
                    nc.vector.tensor_scalar_mul(wrow[:], wrow[:], rec[:])
                    nc.sync.dma_start(wts_out[b:b + 1, :], wrow[:])

                    wtp = tpsps.tile([P, NT], F32, tag="tps")
                    for t in range(NT):
                        nc.tensor.matmul(
                            wtp[:, t:t + 1],
                            wrow[:, t * P:(t + 1) * P],
                            ident[:1, :1],
                            is_transpose=True,
                            start=(t == 0),
                            stop=(t == NT - 1),
                        )
                    wT = rows2.tile([P, NT], F32R, tag="wT")
                    nc.vector.tensor_copy(wT[:], wtp[:])

                    crow = rows2.tile([1, D], F32, tag="crow")
                    for c in range(D // SCH):
                        cxp = ctxps.tile([1, SCH], F32, tag="ctx")
                        for t in range(NT):
                            nc.tensor.matmul(
                                cxp[:],
                                wT[:, t:t + 1],
                                enc_t[t][:, c * SCH:(c + 1) * SCH],
                                start=(t == 0),
                                stop=(t == NT - 1),
                            )
                        nc.vector.tensor_copy(
                            crow[:, c * SCH:(c + 1) * SCH], cxp[:]
                        )
                    nc.sync.dma_start(ctx_out[b:b + 1, :], crow[:])

                def batch_body(b):
                    enc_t = []
                    for t in range(NT):
                        et = encp.tile([P, D], F32R, tag="enc")
                        nc.sync.dma_start(et[:], enc[b, t * P:(t + 1) * P, :])
                        enc_t.append(et)

                    scrow = rows2.tile([1, S], F32, tag="scrow")
                    pending = None
                    for n in range(NCH):
                        # transpose enc chunk -> encT (8 x [128, 512])
                        eT = []
                        for j in range(KJ):
                            tp = tpsps.tile([P, SCH], F32R, tag="tps")
                            for tt in range(SCH // P):
                                t = (SCH // P) * n + tt
                                nc.tensor.matmul(
                                    tp[:, tt * P:(tt + 1) * P],
                                    enc_t[t][:, j * P:(j + 1) * P],
                                    identr[:],
                                    is_transpose=True,
                                    start=(tt == 0),
                                    stop=(tt == SCH // P - 1),
                                )
                            ej = encTp.tile([P, SCH], F32R, tag="encT")
                            nc.vector.tensor_copy(ej[:], tp[:])
                            eT.append(ej)

                        scp = scps.tile([1, SCH], F32, tag="sc")
                        for m in range(MT):
                            kp = keysps.tile([P, SCH], F32, tag="keys")
                            for j in range(KJ):
                                nc.tensor.matmul(
                                    kp[:],
                                    wk_sb[:, j, m * P:(m + 1) * P],
                                    eT[j][:],
                                    start=(j == 0),
                                    stop=(j == KJ - 1),
                                )
                            th = tanhp.tile([P, SCH], F32R, tag="tanh")
                            nc.scalar.activation(
                                th[:], kp[:], AF.Tanh,
                                bias=q_sb[:, m, b:b + 1], scale=1.0,
                            )
                            # scores matmul emitted one m-group late so PE
                            # never waits on the tanh just issued; the copy of
                            # a finished chunk's scores row rides along.
                            if pending is not None:
                                pm, pth, pscp, pn = pending
                                nc.tensor.matmul(
                                    pscp[:], ws_sb[:, pm:pm + 1], pth[:],
                                    start=(pm == 0), stop=(pm == MT - 1),
                                )
                                if pm == MT - 1:
                                    nc.vector.tensor_copy(
                                        scrow[:, pn * SCH:(pn + 1) * SCH],
                                        pscp[:],
                                    )
                            pending = (m, th, scp, n)
                        # interleave the previous batch's tail after chunk 0
                        if n == 0 and state.get("prev") is not None:
                            pb, penc, pscrow = state["prev"]
                            emit_tail(pb, penc, pscrow)
                    pm, pth, pscp, pn = pending
                    nc.tensor.matmul(
                        pscp[:], ws_sb[:, pm:pm + 1], pth[:],
                        start=(pm == 0), stop=(pm == MT - 1),
                    )
                    nc.vector.tensor_copy(
                        scrow[:, pn * SCH:(pn + 1) * SCH], pscp[:]
                    )
                    state["prev"] = (b, enc_t, scrow)


                def all_batches():
                    state["prev"] = None
                    for b in range(BSH):
                        batch_body(b)
                    pb, penc, pscrow = state["prev"]
                    emit_tail(pb, penc, pscrow)
                    state["prev"] = None

                if reps == 1:
                    all_batches()
                else:
                    with tc.For_i(0, reps, 1):
                        all_batches()

    nc.compile()
    return nc


_CACHED_NC = None


def _get_nc():
    global _CACHED_NC
    if _CACHED_NC is None:
        _CACHED_NC = build_nc()
    return _CACHED_NC


def _shard_inputs(decoder_hidden, encoder_outputs, mask, w_query, w_key, w_score):
    dh = np.ascontiguousarray(decoder_hidden, dtype=np.float32)
    enc = np.ascontiguousarray(encoder_outputs, dtype=np.float32)
    mk = np.ascontiguousarray(mask).view(np.uint8).reshape(B_FULL, S)
    wq = np.ascontiguousarray(w_query, dtype=np.float32)
    wk = np.ascontiguousarray(w_key, dtype=np.float32)
    ws = np.ascontiguousarray(w_score, dtype=np.float32)
    in_maps = []
    for c in range(NCORES):
        sl = slice(c * BSH, (c + 1) * BSH)
        in_maps.append(
            {
                "dh": np.ascontiguousarray(dh[sl]),
                "enc": np.ascontiguousarray(enc[sl]),
                "msk": np.ascontiguousarray(mk[sl]),
                "wq": wq,
                "wk": wk,
                "ws": ws,
            }
        )
    return in_maps


def kernel(decoder_hidden, encoder_outputs, mask, w_query, w_key, w_score):
    nc = _get_nc()
    in_maps = _shard_inputs(
        decoder_hidden, encoder_outputs, mask, w_query, w_key, w_score
    )
    res = run_bass_kernel_spmd(nc, in_maps, core_ids=list(range(NCORES)))
    context = np.concatenate([res.results[c]["ctx"] for c in range(NCORES)], axis=0)
    weights = np.concatenate([res.results[c]["wts"] for c in range(NCORES)], axis=0)
    return context, weights


# revision 9
# speedup vs baseline: 1.6454x; 1.6454x over previous
"""Additive (Bahdanau) attention TRN2 Bass kernel.

Problem: nn_AdditiveAttention (B=64, S=2048, DH=DE=DA=1024, fp32).

    query  = decoder_hidden @ w_query            [B, DA]
    keys   = encoder_outputs @ w_key             [B, S, DA]
    scores = tanh(query[:,None,:] + keys) @ w_score   [B, S]
    weights = softmax(where(mask, scores, -1e9))      [B, S]
    context = weights @ encoder_outputs               [B, DE]

Sharding: data-parallel over batch. 8 cores x 8 batches/core; weights
replicated. No collectives.

Per-core plan (all matmuls in float32r = fp32 data, bf16-class PE rate):
  - keysT[DA, S] = w_key^T @ enc^T per batch, via PE on-chip transposes of
    enc (fp32 can't DMA-transpose). lhsT = w_key (natural layout).
  - tanh fused with +query on ACT (query chunk is a per-partition bias).
  - scores = w_score . tanh as an M=1 matmul over the DA partition dim.
  - softmax on a [1, S] row; exp+sum fused in one ACT op.
  - context via M=1 matmuls with naturally-laid-out enc tiles (K=S).
"""

import numpy as np

import concourse.bacc as bacc
import concourse.mybir as mybir
import concourse.tile as tile
from concourse.bass_utils import run_bass_kernel_spmd
from concourse.masks import make_identity

P = 128
B_FULL, S, D = 64, 2048, 1024
NCORES = 8
BSH = B_FULL // NCORES  # batches per core
NT = S // P             # s-tiles per batch (16)
KJ = D // P             # 128-row contraction blocks (8)
SCH = 512               # S-chunk width for keys/scores
NCH = S // SCH          # 4
MT = D // P             # DA m-tiles (8)
F32 = mybir.dt.float32
F32R = mybir.dt.float32r
U8 = mybir.dt.uint8
AF = mybir.ActivationFunctionType
ALU = mybir.AluOpType


def _r(ap):
    """View an fp32 AP as float32r for full-rate PE matmuls."""
    return ap.bitcast(F32R)


def build_nc(reps: int = 1):
    nc = bacc.Bacc("TRN2", target_bir_lowering=False, debug=False)
    dh = nc.dram_tensor("dh", [BSH, D], F32, kind="ExternalInput").ap()
    enc = nc.dram_tensor("enc", [BSH, S, D], F32R, kind="ExternalInput").ap()
    msk = nc.dram_tensor("msk", [BSH, S], U8, kind="ExternalInput").ap()
    wq = nc.dram_tensor("wq", [D, D], F32R, kind="ExternalInput").ap()
    wk = nc.dram_tensor("wk", [D, D], F32R, kind="ExternalInput").ap()
    ws = nc.dram_tensor("ws", [D], F32R, kind="ExternalInput").ap()
    ctx_out = nc.dram_tensor("ctx", [BSH, D], F32, kind="ExternalOutput").ap()
    wts_out = nc.dram_tensor("wts", [BSH, S], F32, kind="ExternalOutput").ap()

    with tile.TileContext(nc) as tc:
        with tc.tile_pool(name="const", bufs=1) as constp:
            ident = constp.tile([P, P], F32)
            make_identity(nc, ident[:])
            identr = constp.tile([P, P], F32R)
            nc.vector.tensor_copy(identr[:], ident[:])
            negbig = constp.tile([1, 1], F32)
            nc.gpsimd.memset(negbig[:], -1e9)

            wk_sb = constp.tile([P, KJ, D], F32R)
            nc.sync.dma_start(wk_sb[:], wk.rearrange("(ko ki) a -> ki ko a", ki=P))
            ws_sb = constp.tile([P, KJ], F32R)
            nc.sync.dma_start(ws_sb[:], ws.rearrange("(ko ki) -> ki ko", ki=P))
            dh_sb = constp.tile([BSH, D], F32)
            nc.sync.dma_start(dh_sb[:], dh)
            dhT_sb = constp.tile([P, KJ, BSH], F32R)
            q_sb = constp.tile([P, MT, BSH], F32)

            # ---- phase 0: Q^T[DA, B] = w_query^T @ dh^T ----
            with (
                tc.tile_pool(name="ph0", bufs=1) as ph0,
                tc.tile_pool(name="ph0ps", bufs=2, space="PSUM") as ph0ps,
            ):
                for k in range(KJ):
                    tp = ph0ps.tile([P, BSH], F32, tag="dhtp")
                    nc.tensor.transpose(
                        tp[:], dh_sb[:, k * P:(k + 1) * P], ident[:BSH, :BSH]
                    )
                    nc.vector.tensor_copy(dhT_sb[:, k, :], tp[:])
                wq_sb = ph0.tile([P, KJ, D], F32R, tag="wq")
                nc.sync.dma_start(wq_sb[:], wq.rearrange("(ko ki) a -> ki ko a", ki=P))
                for m in range(MT):
                    qp = ph0ps.tile([P, BSH], F32, tag="qps")
                    for k in range(KJ):
                        nc.tensor.matmul(
                            qp[:],
                            wq_sb[:, k, m * P:(m + 1) * P],
                            dhT_sb[:, k, :],
                            start=(k == 0),
                            stop=(k == KJ - 1),
                        )
                    nc.vector.tensor_copy(q_sb[:, m, :], qp[:])

            # ---- main per-batch pipeline ----
            with (
                tc.tile_pool(name="encp", bufs=21) as encp,
                tc.tile_pool(name="encTp", bufs=16) as encTp,
                tc.tile_pool(name="tanhp", bufs=4) as tanhp,
                tc.tile_pool(name="rows1", bufs=1) as rows1,
                tc.tile_pool(name="rows2", bufs=2) as rows2,
                tc.tile_pool(name="stats", bufs=2) as statsp,
                tc.tile_pool(name="keysps", bufs=2, space="PSUM") as keysps,
                tc.tile_pool(name="tpsps", bufs=3, space="PSUM") as tpsps,
                tc.tile_pool(name="scps", bufs=2, space="PSUM") as scps,
                tc.tile_pool(name="ctxps", bufs=1, space="PSUM") as ctxps,
            ):

                def batch_body(b):
                    enc_t = []
                    for t in range(NT):
                        et = encp.tile([P, D], F32R, tag="enc")
                        nc.sync.dma_start(et[:], enc[b, t * P:(t + 1) * P, :])
                        enc_t.append(et)

                    mrow = rows1.tile([1, S], U8, tag="mrow")
                    nc.sync.dma_start(mrow[:], msk[b:b + 1, :])
                    notm = rows1.tile([1, S], U8, tag="notm")
                    nc.vector.tensor_scalar(
                        notm[:], mrow[:], 0, None, ALU.is_equal
                    )

                    scrow = rows2.tile([1, S], F32, tag="scrow")
                    pending = None
                    for n in range(NCH):
                        # transpose enc chunk -> encT (8 x [128, 512])
                        eT = []
                        for j in range(KJ):
                            tp = tpsps.tile([P, SCH], F32R, tag="tps")
                            for tt in range(SCH // P):
                                t = (SCH // P) * n + tt
                                nc.tensor.matmul(
                                    tp[:, tt * P:(tt + 1) * P],
                                    enc_t[t][:, j * P:(j + 1) * P],
                                    identr[:],
                                    is_transpose=True,
                                    start=(tt == 0),
                                    stop=(tt == SCH // P - 1),
                                )
                            ej = encTp.tile([P, SCH], F32R, tag="encT")
                            nc.vector.tensor_copy(ej[:], tp[:])
                            eT.append(ej)

                        scp = scps.tile([1, SCH], F32, tag="sc")
                        for m in range(MT):
                            kp = keysps.tile([P, SCH], F32, tag="keys")
                            for j in range(KJ):
                                nc.tensor.matmul(
                                    kp[:],
                                    wk_sb[:, j, m * P:(m + 1) * P],
                                    eT[j][:],
                                    start=(j == 0),
                                    stop=(j == KJ - 1),
                                )
                            th = tanhp.tile([P, SCH], F32R, tag="tanh")
                            nc.scalar.activation(
                                th[:], kp[:], AF.Tanh,
                                bias=q_sb[:, m, b:b + 1], scale=1.0,
                            )
                            if pending is not None:
                                pm, pth, pscp, pn = pending
                                nc.tensor.matmul(
                                    pscp[:], ws_sb[:, pm:pm + 1], pth[:],
                                    start=(pm == 0), stop=(pm == MT - 1),
                                )
                                if pm == MT - 1:
                                    nc.vector.tensor_copy(
                                        scrow[:, pn * SCH:(pn + 1) * SCH],
                                        pscp[:],
                                    )
                            pending = (m, th, scp, n)
                    pm, pth, pscp, pn = pending
                    nc.tensor.matmul(
                        pscp[:], ws_sb[:, pm:pm + 1], pth[:],
                        start=(pm == 0), stop=(pm == MT - 1),
                    )
                    nc.vector.tensor_copy(
                        scrow[:, pn * SCH:(pn + 1) * SCH], pscp[:]
                    )

                    # mask + softmax on the [1, S] score row
                    nc.vector.copy_predicated(
                        scrow[:], notm[:], negbig[:].to_broadcast((1, S))
                    )
                    mx = statsp.tile([1, 1], F32, tag="mx")
                    nc.vector.reduce_max(
                        mx[:], scrow[:], axis=mybir.AxisListType.X, negate=True
                    )
                    wrow = rows1.tile([1, S], F32, tag="wrow")
                    sm = statsp.tile([1, 1], F32, tag="sm")
                    nc.scalar.activation(
                        wrow[:], scrow[:], AF.Exp, bias=mx[:], scale=1.0,
                        accum_out=sm[:],
                    )
                    rec = statsp.tile([1, 1], F32, tag="rec")
                    nc.vector.reciprocal(rec[:], sm[:])
                    nc.vector.tensor_scalar_mul(wrow[:], wrow[:], rec[:])
                    nc.sync.dma_start(wts_out[b:b + 1, :], wrow[:])

                    # transpose weights row -> [S(part), NT] column form
                    wtp = tpsps.tile([P, NT], F32, tag="tps")
                    for t in range(NT):
                        nc.tensor.matmul(
                            wtp[:, t:t + 1],
                            wrow[:, t * P:(t + 1) * P],
                            ident[:1, :1],
                            is_transpose=True,
                            start=(t == 0),
                            stop=(t == NT - 1),
                        )
                    wT = rows2.tile([P, NT], F32R, tag="wT")
                    nc.vector.tensor_copy(wT[:], wtp[:])

                    # context = weights @ enc (K = S on partitions)
                    crow = rows2.tile([1, D], F32, tag="crow")
                    for c in range(D // SCH):
                        cxp = ctxps.tile([1, SCH], F32, tag="ctx")
                        for t in range(NT):
                            nc.tensor.matmul(
                                cxp[:],
                                wT[:, t:t + 1],
                                enc_t[t][:, c * SCH:(c + 1) * SCH],
                                start=(t == 0),
                                stop=(t == NT - 1),
                            )
                        nc.vector.tensor_copy(
                            crow[:, c * SCH:(c + 1) * SCH], cxp[:]
                        )
                    nc.sync.dma_start(ctx_out[b:b + 1, :], crow[:])

                if reps == 1:
                    for b in range(BSH):
                        batch_body(b)
                else:
                    with tc.For_i(0, reps, 1):
                        for b in range(BSH):
                            batch_body(b)

    nc.compile()
    return nc


_CACHED_NC = None


def _get_nc():
    global _CACHED_NC
    if _CACHED_NC is None:
        _CACHED_NC = build_nc()
    return _CACHED_NC


def _shard_inputs(decoder_hidden, encoder_outputs, mask, w_query, w_key, w_score):
    dh = np.ascontiguousarray(decoder_hidden, dtype=np.float32)
    enc = np.ascontiguousarray(encoder_outputs, dtype=np.float32)
    mk = np.ascontiguousarray(mask).view(np.uint8).reshape(B_FULL, S)
    wq = np.ascontiguousarray(w_query, dtype=np.float32)
    wk = np.ascontiguousarray(w_key, dtype=np.float32)
    ws = np.ascontiguousarray(w_score, dtype=np.float32)
    in_maps = []
    for c in range(NCORES):
        sl = slice(c * BSH, (c + 1) * BSH)
        in_maps.append(
            {
                "dh": np.ascontiguousarray(dh[sl]),
                "enc": np.ascontiguousarray(enc[sl]),
                "msk": np.ascontiguousarray(mk[sl]),
                "wq": wq,
                "wk": wk,
                "ws": ws,
            }
        )
    return in_maps


def kernel(decoder_hidden, encoder_outputs, mask, w_query, w_key, w_score):
    nc = _get_nc()
    in_maps = _shard_inputs(
        decoder_hidden, encoder_outputs, mask, w_query, w_key, w_score
    )
    res = run_bass_kernel_spmd(nc, in_maps, core_ids=list(range(NCORES)))
    context = np.concatenate([res.results[c]["ctx"] for c in range(NCORES)], axis=0)
    weights = np.concatenate([res.results[c]["wts"] for c in range(NCORES)], axis=0)
    return context, weights
